# revision 18
# baseline (speedup 1.0000x reference)
"""CFSDP (density-peaks clustering) on 8 Trainium2 NeuronCores.

Pipeline (N=8192 points, D=64, row-sharded 1024 rows/core):
  d2(i,j) = ||xi-xj||^2 via one K=66 augmented matmul per tile:
      u_i = (-2*x_i, sq_i, 1),  v_j = (x_j, 1, sq_j),  d2 = u_i . v_j
  All O(N^2) math runs on squared distances (sqrt is monotone, so order
  stats / argmin / percentile commute with it):
    L1: count(d2 < t_b) for 16 thresholds around the predicted 2%-quantile
        (ACT sigmoid step fn + accumulate) -> host interpolates dc^2.
    L2: rho_i = sum_j exp(-d2_ij/dc^2) (ACT Exp + accumulate, scale from SBUF).
    host: stable-sort rows by rho desc; "higher density" mask becomes a
        per-row prefix of the sorted column order.
    L3: delta_i^2 = min over prefix window of d2 (vector.tensor_mask_reduce,
        per-partition index window, on negated-d2 PSUM tiles).
  Host finishes: delta fallback (row max) for top-density rows, nhd argmin
  (lazy, only for non-center points), center ranks, label propagation scan.
"""

import os
import numpy as np

N = 8192
D = 64
NCORES = 8
ROWS = N // NCORES          # 1024 rows per core
P = 128                     # partitions
RB = ROWS // P              # 8 row-blocks per core
FD = 2048                   # free-dim group (4 PSUM banks)
G = N // FD                 # 4 col-groups per row
K = D + 4                   # 68 (augmented contraction dim, sq split hi+lo)
MM_N = 512                  # cols per matmul (one PSUM bank output)
MM_PER_G = FD // MM_N       # 4

NT = 4                      # percentile-count thresholds
L1_W = 1024                 # cols counted per threshold
DC2_CENTER = 86.2           # chi^2_64-predicted 2%-quantile of d2 (randn data)
DC2_GRID = (DC2_CENTER * (1.0 + (np.arange(NT) - (NT - 1) / 2) * 0.023)).astype(
    np.float64
)                           # +-3.5% bracket, 2.3% spacing
SIG_ALPHA = 2.0e4           # sigmoid step sharpness (soft window ~1e-3 in d2)
PCT = 2.0
FLT_MAX = float(np.finfo(np.float32).max)
PEN_BIG = 1e38              # penalty added beyond the prefix cutoff
PEN_ALPHA = 1e31            # relu penalty slope (ACT-built mask, L3)
WW = 1024                   # L3 boundary mask window width
NCOL = G + 1                # L3 output cols per block (G group slots + window)
EMPTY_SENTINEL = 1e37       # accum >= this => empty prefix window

# threshold b is counted on group (m, g) of every core (1/16 of the matrix
# per threshold => ~4.2M samples each; different rows+cols per threshold)
L1_GROUPS = [(b % RB, 1 + b % (G - 1)) for b in range(NT)]  # g>0: diag-free
DC2_STEP = float(DC2_CENTER * 0.023)
M_TOT = float(N) * float(N)
K_POS = PCT / 100.0 * (M_TOT - 1.0)
P_OFF = (K_POS - N) / (M_TOT - N)      # diag-free target CDF
CSTAR = float(P_OFF * P * L1_W)        # target count over the device sample

_programs: dict = {}


def _f32r(ap):
    import concourse.mybir as mybir

    return ap.bitcast(mybir.dt.float32r)



def _pe_warmup(nc, tc, inp, psum_p, mybir, n_mm=8):
    """Dense garbage-matmul burst at launch start: runs while the input DMA
    streams, trips the PE HAM un-throttle (~3.4us sustained busy) so real
    matmuls run at 2.4 GHz instead of 1.2 GHz."""
    f32 = mybir.dt.float32
    warm = inp.tile([K, P + MM_N], mybir.dt.bfloat16)
    nc.gpsimd.memset(warm[:], 1.0)
    wps = psum_p.tile([P, FD], f32, tag="psum")
    for j in range(n_mm):
        nc.tensor.matmul(
            wps[:, (j % MM_PER_G) * MM_N:((j % MM_PER_G) + 1) * MM_N],
            warm[:, :P],
            warm[:, P:P + MM_N],
            start=True,
            stop=True,
        )


def _build_l12():
    """Merged count + rho launch: dc^2 is computed ON DEVICE.

    Every core counts the SAME sample (rows 0..1023 via the shared `uvc`
    lhsT, diag-free col groups), so each core independently derives an
    identical dc^2 - no collectives. The CDF interpolation runs as tiny
    [1,8] vector ops; a PE ones-matmul does the partition reduction and a
    K=1 fp32 matmul broadcasts -1/dc^2 to all partitions for the rho phase.
    `dvec` carries host-computed control-variate corrections (in counts)
    that cancel the row/col sampling bias of the fixed sample.
    """
    import concourse.mybir as mybir
    import concourse.tile as tile
    from concourse import bacc

    f32 = mybir.dt.float32
    nc = bacc.Bacc("TRN2", debug=False, enable_asserts=False)
    bf16 = mybir.dt.bfloat16
    uv_d = nc.dram_tensor("uv", [K, ROWS + N], bf16, kind="ExternalInput")
    uvc_d = nc.dram_tensor("uvc", [K, ROWS], bf16, kind="ExternalInput")
    thr_d = nc.dram_tensor("thr", [P, NT], f32, kind="ExternalInput")
    tvec_d = nc.dram_tensor("tvec", [1, NT], f32, kind="ExternalInput")
    dvec_d = nc.dram_tensor("dvec", [1, NT], f32, kind="ExternalInput")
    cnt_d = nc.dram_tensor("counts", [P, NT], f32, kind="ExternalOutput")
    rho_d = nc.dram_tensor("rho", [P, RB], f32, kind="ExternalOutput")

    with tile.TileContext(nc) as tc:
        with (
            tc.tile_pool(name="inp", bufs=1) as inp,
            tc.tile_pool(name="stat", bufs=1) as stat,
            tc.tile_pool(name="trash", bufs=2) as trash_p,
            tc.tile_pool(name="psum", bufs=2, space="PSUM") as psum_p,
        ):
            uv_sb = inp.tile([K, ROWS + N], bf16)
            uvc_sb = inp.tile([K, ROWS], bf16)
            nc.sync.dma_start(out=uvc_sb[:], in_=uvc_d[:])
            for _g in (1, 2, 3):  # count-phase cols first; group 0 only for rho
                _a = ROWS + _g * FD
                nc.sync.dma_start(
                    out=uv_sb[:, _a:_a + FD], in_=uv_d[:, _a:_a + FD]
                )
            nc.sync.dma_start(out=uv_sb[:, 0:ROWS], in_=uv_d[:, 0:ROWS])
            nc.sync.dma_start(
                out=uv_sb[:, ROWS:ROWS + FD], in_=uv_d[:, ROWS:ROWS + FD]
            )
            thr_sb = inp.tile([P, NT], f32)
            nc.gpsimd.dma_start(out=thr_sb[:], in_=thr_d[:])
            tdv_sb = inp.tile([1, 2 * NT], f32)
            nc.gpsimd.dma_start(out=tdv_sb[:, 0:NT], in_=tvec_d[:])
            nc.gpsimd.dma_start(out=tdv_sb[:, NT:2 * NT], in_=dvec_d[:])
            cnts = stat.tile([P, NT], f32)
            warmact = stat.tile([P, 1], f32)
            nc.scalar.activation(
                warmact[:], thr_sb[:, 0:1],
                mybir.ActivationFunctionType.Sigmoid, bias=0.0, scale=1.0,
            )

            # ---- phase 1: counts over the shared sample -----------------
            for b, (m, g) in enumerate(L1_GROUPS):
                psum = psum_p.tile([P, FD], f32, tag="psum")
                for j in range(L1_W // MM_N):
                    nc.tensor.matmul(
                        psum[:, j * MM_N:(j + 1) * MM_N],
                        uvc_sb[:, m * P:(m + 1) * P],
                        uv_sb[:, ROWS + g * FD + j * MM_N: ROWS + g * FD + (j + 1) * MM_N],
                        start=True,
                        stop=True,
                    )
                t = trash_p.tile([P, L1_W], f32, tag="cntrash")
                nc.scalar.activation(
                    t[:],
                    psum[:, 0:L1_W],
                    mybir.ActivationFunctionType.Sigmoid,
                    bias=thr_sb[:, b:b + 1],
                    scale=float(-SIG_ALPHA),
                    accum_out=cnts[:, b:b + 1],
                )
            nc.gpsimd.dma_start(out=cnt_d[:], in_=cnts[:])

            # ---- phase 2: dc^2 from counts (identical on every core) ----
            ones_col = stat.tile([P, 1], f32)
            nc.vector.memset(ones_col[:], 1.0)
            ps_tot = psum_p.tile([1, NT], f32, tag="psum")
            nc.tensor.matmul(ps_tot[:], ones_col[:], cnts[:], start=True, stop=True)
            w = stat.tile([1, 8 * NT], f32)  # scratch lanes along free dim
            q = w[:, 0:NT]
            nc.vector.tensor_tensor(
                out=q, in0=ps_tot[:], in1=tdv_sb[:, NT:2 * NT],
                op=mybir.AluOpType.subtract,
            )
            NB_ = NT - 1
            a_ = w[:, NT:NT + NB_]
            nc.vector.tensor_scalar(
                out=a_, in0=q[:, 0:NB_], scalar1=CSTAR, scalar2=None,
                op0=mybir.AluOpType.is_le,
            )
            b_ = w[:, 2 * NT:2 * NT + NB_]
            nc.vector.tensor_scalar(
                out=b_, in0=q[:, 1:NT], scalar1=CSTAR, scalar2=None,
                op0=mybir.AluOpType.is_gt,
            )
            sel = w[:, 3 * NT:3 * NT + NB_]
            nc.vector.tensor_tensor(out=sel, in0=a_, in1=b_, op=mybir.AluOpType.mult)
            den = w[:, 4 * NT:4 * NT + NB_]
            nc.vector.tensor_tensor(
                out=den, in0=q[:, 1:NT], in1=q[:, 0:NB_],
                op=mybir.AluOpType.subtract,
            )
            rec = w[:, 5 * NT:5 * NT + NB_]
            nc.vector.reciprocal(rec, den)
            num = w[:, 6 * NT:6 * NT + NB_]
            nc.vector.tensor_scalar(
                out=num, in0=q[:, 0:NB_], scalar1=-1.0, scalar2=CSTAR,
                op0=mybir.AluOpType.mult, op1=mybir.AluOpType.add,
            )
            fr = w[:, 7 * NT:7 * NT + NB_]
            nc.vector.tensor_tensor(out=fr, in0=num, in1=rec, op=mybir.AluOpType.mult)
            nc.vector.tensor_scalar(
                out=fr, in0=fr, scalar1=float(DC2_STEP), scalar2=None,
                op0=mybir.AluOpType.mult,
            )
            nc.vector.tensor_tensor(
                out=fr, in0=fr, in1=tdv_sb[:, 0:NB_], op=mybir.AluOpType.add
            )
            nc.vector.tensor_tensor(out=fr, in0=fr, in1=sel, op=mybir.AluOpType.mult)
            sc = stat.tile([1, 4], f32)
            nc.vector.tensor_reduce(
                sc[:, 0:1], fr[:], axis=mybir.AxisListType.X, op=mybir.AluOpType.add
            )
            nc.vector.tensor_reduce(
                sc[:, 1:2], sel[:], axis=mybir.AxisListType.X, op=mybir.AluOpType.add
            )
            # guard: if no bracket, fall back to the grid center
            nc.vector.tensor_scalar(
                out=sc[:, 2:3], in0=sc[:, 1:2], scalar1=float(-DC2_CENTER),
                scalar2=float(DC2_CENTER), op0=mybir.AluOpType.mult,
                op1=mybir.AluOpType.add,
            )
            nc.vector.tensor_tensor(
                out=sc[:, 0:1], in0=sc[:, 0:1], in1=sc[:, 2:3],
                op=mybir.AluOpType.add,
            )
            nc.vector.reciprocal(sc[:, 3:4], sc[:, 0:1])
            nc.vector.tensor_scalar(
                out=sc[:, 3:4], in0=sc[:, 3:4], scalar1=-1.0, scalar2=None,
                op0=mybir.AluOpType.mult,
            )
            ones_row = stat.tile([1, P], f32)
            nc.vector.memset(ones_row[:], 1.0)
            ps_b = psum_p.tile([P, 1], f32, tag="psum")
            nc.tensor.matmul(ps_b[:], ones_row[:], sc[:, 3:4], start=True, stop=True)
            scl_sb = stat.tile([P, 1], f32)
            nc.vector.tensor_copy(scl_sb[:], ps_b[:])

            # ---- phase 3: rho ------------------------------------------
            parts = stat.tile([P, RB * G], f32)
            rho_sb = stat.tile([P, RB], f32)
            for m in range(RB):
                for g in range(G):
                    psum = psum_p.tile([P, FD], f32, tag="psum")
                    for j in range(MM_PER_G):
                        nc.tensor.matmul(
                            psum[:, j * MM_N:(j + 1) * MM_N],
                            uv_sb[:, m * P:(m + 1) * P],
                            uv_sb[:, ROWS + g * FD + j * MM_N: ROWS + g * FD + (j + 1) * MM_N],
                            start=True,
                            stop=True,
                        )
                    t = trash_p.tile([P, FD], f32, tag="trash")
                    q2 = m * G + g
                    nc.scalar.activation(
                        t[:],
                        psum[:],
                        mybir.ActivationFunctionType.Exp,
                        bias=0.0,
                        scale=scl_sb[:, 0:1],
                        accum_out=parts[:, q2:q2 + 1],
                    )
                nc.vector.tensor_reduce(
                    rho_sb[:, m:m + 1],
                    parts[:, m * G:(m + 1) * G],
                    axis=mybir.AxisListType.X,
                    op=mybir.AluOpType.add,
                )
            nc.sync.dma_start(out=rho_d[:], in_=rho_sb[:])
    nc.compile()
    return nc


def _build_l3():
    """Delta pass on rho-sorted data (round-robin block interleaving).

    Core c holds sorted row-blocks b = 8m + c (m = 0..7). For local block m:
      boundary col-group g_b = m//2, window base w_lo = 1024*(m%2)
      (cutoffs of every core's block-m rows lie in [w_lo, w_lo+1024) of
      group g_b, ties aside - those are patched on host).
    Structure per block:
      groups g < g_b:                plain min-reduce of the whole group
      boundary prefix [0, w_lo):     plain min-reduce (odd m only)
      boundary window [w_lo,+1024):  penalty mask (iota >= cutrel)*BIG, add,
                                     min-reduce
      columns beyond w_lo+1024 and groups g > g_b: skipped entirely.
    """
    import concourse.mybir as mybir
    import concourse.tile as tile
    from concourse import bacc

    f32 = mybir.dt.float32
    nc = bacc.Bacc("TRN2", debug=False, enable_asserts=False)
    bf16 = mybir.dt.bfloat16
    uv_d = nc.dram_tensor("uv", [K, ROWS + N], bf16, kind="ExternalInput")
    cut_d = nc.dram_tensor("cut", [P, RB], f32, kind="ExternalInput")
    iota_d = nc.dram_tensor("iota", [P, WW], f32, kind="ExternalInput")
    dmin_d = nc.dram_tensor("dmin", [P, RB * NCOL], f32, kind="ExternalOutput")

    with tile.TileContext(nc) as tc:
        with (
            tc.tile_pool(name="inp", bufs=1) as inp,
            tc.tile_pool(name="stat", bufs=1) as stat,
            tc.tile_pool(name="trash", bufs=3) as trash_p,
            tc.tile_pool(name="pen", bufs=3) as pen_p,
            tc.tile_pool(name="psum", bufs=2, space="PSUM") as psum_p,
        ):
            uv_sb = inp.tile([K, ROWS + N], bf16)
            nc.sync.dma_start(out=uv_sb[:, 0:ROWS], in_=uv_d[:, 0:ROWS])
            for _g in range(G):
                _a = ROWS + _g * FD
                nc.sync.dma_start(
                    out=uv_sb[:, _a:_a + FD], in_=uv_d[:, _a:_a + FD]
                )
            cut_sb = inp.tile([P, RB], f32)
            nc.gpsimd.dma_start(out=cut_sb[:], in_=cut_d[:])
            iota_sb = inp.tile([P, WW], f32)
            nc.gpsimd.dma_start(out=iota_sb[:], in_=iota_d[:])
            dmin_sb = stat.tile([P, RB * NCOL], f32)

            for m in range(RB):
                gb = m // 2
                w_lo = WW * (m % 2)
                pen = pen_p.tile([P, WW], f32, tag="pen")
                # cutrel (host-clamped to [0, WW]) is relative to w_lo
                nc.vector.tensor_scalar(
                    out=pen[:],
                    in0=iota_sb[:],
                    scalar1=cut_sb[:, m:m + 1],
                    scalar2=PEN_BIG,
                    op0=mybir.AluOpType.is_ge,
                    op1=mybir.AluOpType.mult,
                )
                for g in range(gb + 1):
                    ncols = FD if g < gb else w_lo + WW
                    psum = psum_p.tile([P, FD], f32, tag="psum")
                    for j in range(ncols // MM_N):
                        nc.tensor.matmul(
                            psum[:, j * MM_N:(j + 1) * MM_N],
                            uv_sb[:, m * P:(m + 1) * P],
                            uv_sb[:, ROWS + g * FD + j * MM_N: ROWS + g * FD + (j + 1) * MM_N],
                            start=True,
                            stop=True,
                        )
                    q = m * NCOL + g
                    if g < gb:
                        nc.vector.tensor_reduce(
                            dmin_sb[:, q:q + 1],
                            psum[:],
                            axis=mybir.AxisListType.X,
                            op=mybir.AluOpType.min,
                        )
                    else:
                        if w_lo > 0:
                            nc.vector.tensor_reduce(
                                dmin_sb[:, q:q + 1],
                                psum[:, 0:w_lo],
                                axis=mybir.AxisListType.X,
                                op=mybir.AluOpType.min,
                            )
                        t = trash_p.tile([P, WW], f32, tag="trash")
                        nc.vector.tensor_tensor(
                            out=t[:],
                            in0=psum[:, w_lo:w_lo + WW],
                            in1=pen[:],
                            op=mybir.AluOpType.add,
                        )
                        nc.vector.tensor_reduce(
                            dmin_sb[:, m * NCOL + G:m * NCOL + G + 1],
                            t[:],
                            axis=mybir.AxisListType.X,
                            op=mybir.AluOpType.min,
                        )
            nc.gpsimd.dma_start(out=dmin_d[:], in_=dmin_sb[:])
    nc.compile()
    return nc


CK = 66                     # bf16 diag GEMM contraction: 64 x + sq hi/lo
KI8 = 34                    # fp8 DoubleRow half-contraction (virtual K=68)
C_NACT = 18                 # accumulating ACT (exp-detector) tiles per core
C_NDVE = 18                 # DVE (min-reduce) tiles per core
C_PW = 1024                 # psum tile width (4 rotating buffers)
CERT_EPS8 = 4.6             # fp8 d2 error bound (measured max 4.43)
CERT_SMAX8 = 0.03           # fp8 exp-detector flag threshold (signal >= .09)
CERT_DET8 = 2.4             # un-flagged fp8 pairs have true d2 above this
CERT_EPSD = 0.60            # bf16 diag-tile d2 error bound (self-probe .41)
F8_PAD = 240.0              # sentinel sq value for shifted-out pad columns

def _plan_cert():
    """Per-core tile schedule (identical on every core; SPMD shape).

    Local block m (global rows 128*(8m+c)) covers SHIFTED cols
    [1024m+128, 8192) — per-core V is rotated left by 128c so the strict
    upper triangle starts at the same local offset on every core; the
    128-col diagonal run [1024m, +128) is handled by a separate bf16 diag
    tile. Coverage is cut into [128, <=1024] psum tiles, each consumed
    whole by one engine: 'A' = ACT exp detector (row-sum accumulate),
    'V' = DVE min-reduce, strictly alternating. Tiles descend in column
    start, matching the descending-chunk DMA of V8.
    """
    raw = []
    for m in range(RB):
        raw.append((m, 1024 * m + 128, 896))
        for k in range(m + 1, 8):
            raw.append((m, 1024 * k, 1024))
    raw.sort(key=lambda t: (-t[1], t[0]))
    tiles = []
    s_i = 0
    v_i = 0
    for i, (m, g0, w) in enumerate(raw):
        if i % 2 == 0:
            tiles.append(dict(m=m, g0=g0, w=w, eng="A", col=s_i))
            s_i += 1
        else:
            tiles.append(dict(m=m, g0=g0, w=w, eng="V", col=v_i))
            v_i += 1
    assert s_i == C_NACT and v_i == C_NDVE
    return tiles


def _chunks(w):
    """Split a tile width into <=512-col matmul chunks."""
    out = []
    o = 0
    while o < w:
        out.append((o, min(MM_N, w - o)))
        o += MM_N
    return out


def _build_cert():
    """Single-launch close-pair certificate.

    Main stream: fp8 DoubleRow matmuls (virtual K=68: 64 x dims + 3-way sq_j
    split) produce psum = sq_j - 2 xi.xj for the strict block-upper triangle
    in per-core shifted column space; whole psum tiles go either to the ACT
    exp detector (scale=-1/2, bias=-sq_i/2, row-sum accumulate) or to a DVE
    min-reduce. A separate bf16 tile handles the eight 128x128 diagonal
    sub-blocks; its raw exp values are shipped to the host, which finishes
    the intra-block analysis exactly."""
    import concourse.mybir as mybir
    import concourse.tile as tile
    from concourse import bacc

    f32 = mybir.dt.float32
    bf16 = mybir.dt.bfloat16
    f8 = mybir.dt.float8e4
    nc = bacc.Bacc("TRN2", debug=False, enable_asserts=False)
    # uv8: local V8 (cols 0..N) with the per-core u8 appended at N..N+ROWS
    uv8_d = nc.dram_tensor("uv8", [KI8, 2, N + ROWS], f8, kind="ExternalInput")
    # uvd: bf16 diag operands, u at cols 0..ROWS, vd at ROWS..2*ROWS
    uvd_d = nc.dram_tensor("uvd", [CK, 2 * ROWS], bf16, kind="ExternalInput")
    bias_d = nc.dram_tensor("bias", [P, RB], f32, kind="ExternalInput")
    # out: s sums [0:18], mins [18:36], diag exp tile bf16-packed [36:548]
    out_d = nc.dram_tensor("out", [P, 548], f32, kind="ExternalOutput")

    with tile.TileContext(nc) as tc:
        with (
            tc.tile_pool(name="inp", bufs=1) as inp,
            tc.tile_pool(name="stat", bufs=1) as stat,
            tc.tile_pool(name="trash", bufs=2) as trash_p,
            tc.tile_pool(name="psum", bufs=4, space="PSUM") as psum_p,
        ):
            uv8_sb = inp.tile([KI8, 2, N + ROWS], f8)
            uvd_sb = inp.tile([CK, 2 * ROWS], bf16)
            bias_sb = inp.tile([P, RB], f32)
            out_sb = stat.tile([P, 548], f32)
            wact = stat.tile([P, 2], f32)
            # first slice carries u8 + the high V8 cols the first tiles need
            nc.sync.dma_start(
                out=uv8_sb[:, :, 7168:N + ROWS], in_=uv8_d[:, :, 7168:N + ROWS]
            )
            nc.scalar.dma_start(out=uvd_sb[:], in_=uvd_d[:])
            nc.scalar.dma_start(out=bias_sb[:], in_=bias_d[:])
            nc.sync.dma_start(
                out=uv8_sb[:, :, 3584:7168], in_=uv8_d[:, :, 3584:7168]
            )
            nc.sync.dma_start(out=uv8_sb[:, :, 0:3584], in_=uv8_d[:, :, 0:3584])
            nc.vector.memset(wact[:, 0:1], 0.0)
            nc.scalar.activation(
                wact[:, 1:2], wact[:, 0:1],
                mybir.ActivationFunctionType.Exp, bias=0.0, scale=1.0,
            )
            s_sb = out_sb[:, 0:C_NACT]
            m_sb = out_sb[:, C_NACT:C_NACT + C_NDVE]
            dg_sb = out_sb[:, 36:548].bitcast(bf16)  # [P, ROWS]

            def diag_tile():
                # bf16 diagonal: eight 128x128 sub-blocks, one psum tile
                dps = psum_p.tile([P, C_PW], f32, tag="psum")
                for m in range(RB):
                    nc.tensor.matmul(
                        dps[:, m * P:(m + 1) * P],
                        uvd_sb[:, m * P:(m + 1) * P],
                        uvd_sb[:, ROWS + m * P:ROWS + (m + 1) * P],
                        start=True,
                        stop=True,
                    )
                nc.scalar.activation(
                    dg_sb,
                    dps[:],
                    mybir.ActivationFunctionType.Exp,
                    bias=0.0,
                    scale=-0.5,
                )

            for i, t in enumerate(_plan_cert()):
                if i == 10:
                    diag_tile()  # scalar-DMA'd operands are ready by now
                m, g0, w, col = t["m"], t["g0"], t["w"], t["col"]
                psum = psum_p.tile([P, C_PW], f32, tag="psum")
                for o, cw in _chunks(w):
                    nc.tensor.matmul(
                        psum[:, o:o + cw],
                        uv8_sb[:, :, N + m * P:N + (m + 1) * P],
                        uv8_sb[:, :, g0 + o:g0 + o + cw],
                        start=True,
                        stop=True,
                        perf_mode=mybir.MatmulPerfMode.DoubleRow,
                    )
                if t["eng"] == "A":
                    tr = trash_p.tile([P, C_PW], bf16, tag="trash")
                    nc.scalar.activation(
                        tr[:, 0:w],
                        psum[:, 0:w],
                        mybir.ActivationFunctionType.Exp,
                        bias=bias_sb[:, m:m + 1],
                        scale=-0.5,
                        accum_out=s_sb[:, col:col + 1],
                    )
                else:
                    nc.vector.tensor_reduce(
                        m_sb[:, col:col + 1],
                        psum[:, 0:w],
                        axis=mybir.AxisListType.X,
                        op=mybir.AluOpType.min,
                    )
            nc.sync.dma_start(out=out_d[:], in_=out_sb[:])
    nc.compile()
    return nc


_BUILDERS = {"l12": _build_l12, "l3": _build_l3, "cert": _build_cert}


def _get_program(name):
    if name not in _programs:
        _programs[name] = _BUILDERS[name]()
    return _programs[name]


TIMINGS = []  # (name, exec_time_ns) per launch, appended by _run


def _run(name, in_maps, trace=None):
    from concourse.bass_utils import run_bass_kernel_spmd

    if trace is None:
        trace = bool(int(os.environ.get("KERNEL_TRACE", "0")))
    nc = _get_program(name)
    res = run_bass_kernel_spmd(
        nc, in_maps, core_ids=list(range(NCORES)), trace=trace
    )
    TIMINGS.append((name, res.exec_time_ns))
    return res


def _augmented(data):
    """U (lhs rows) and V (rhs cols) of the K=68 augmented distance GEMM.

    bf16 operands with sq split into a bf16 hi+lo pair: d2 error ~0.04 abs
    (~5e-4 relative at the dc^2 scale), far inside every decision margin.
    """
    import ml_dtypes

    bf = ml_dtypes.bfloat16
    sq = np.einsum("ij,ij->i", data, data, dtype=np.float32).astype(np.float32)
    sqh = sq.astype(bf)
    sql = (sq - sqh.astype(np.float32)).astype(bf)
    ones = np.ones((N, 1), bf)
    zcol = lambda a: a[:, None]
    U = np.concatenate(
        [(-2.0 * data).astype(bf), zcol(sqh), zcol(sql), ones, ones], axis=1
    )
    V = np.concatenate(
        [data.astype(bf), ones, ones, zcol(sqh), zcol(sql)], axis=1
    )
    return U, V, sq


def _erf(x):
    """Abramowitz-Stegun 7.1.26 vectorized erf (|err| < 1.5e-7)."""
    s = np.sign(x)
    x = np.abs(x)
    t = 1.0 / (1.0 + 0.3275911 * x)
    y = 1.0 - (
        ((((1.061405429 * t - 1.453152027) * t) + 1.421413741) * t - 0.284496736)
        * t
        + 0.254829592
    ) * t * np.exp(-x * x)
    return s * y


def _phi(z):
    return 0.5 * (1.0 + _erf(z / np.sqrt(2.0)))


NGRID = 256


def _cv_corrections(sq):
    """Control-variate count corrections for the fixed count sample.

    Model P(d2 < t | sq_i, sq_j) ~ Phi((t - sq_i - sq_j)/(2 sqrt(sq_i sq_j/D)))
    and subtract the predicted row/col selection bias of the sampled
    rows/cols relative to the full point set.
    """
    sq64 = sq.astype(np.float64)
    step = N // NGRID
    grid = np.sort(sq64)[step // 2::step][:NGRID]

    def h(t, svals):
        s = svals[:, None]
        sp = grid[None, :]
        z = (t - s - sp) / (2.0 * np.sqrt(np.maximum(s * sp, 1e-9) / D))
        return _phi(z).mean(axis=1)

    dvec = np.zeros(NT)
    for b, (m, g) in enumerate(L1_GROUPS):
        t = float(DC2_GRID[b])
        h_all = h(t, grid).mean()
        d_row = h(t, sq64[m * P:(m + 1) * P]).mean() - h_all
        d_col = h(t, sq64[g * FD:g * FD + L1_W]).mean() - h_all
        dvec[b] = (d_row + d_col) * (P * L1_W)
    return dvec.astype(np.float32).reshape(1, NT)


def _interp_dc2(counts_by_core):
    """counts_by_core: list of [P, NT] arrays -> dc^2 via CDF interpolation."""
    M = float(N) * float(N)
    k_pos = PCT / 100.0 * (M - 1.0)
    p_off = (k_pos - N) / (M - N)  # diag cells (d2=0) all fall below any t_b

    tot = np.zeros(NT, np.float64)
    denom = np.zeros(NT, np.float64)
    for c in range(NCORES):
        cc = counts_by_core[c].astype(np.float64).sum(axis=0)  # [NT]
        for b, (m, g) in enumerate(L1_GROUPS):
            row0 = c * ROWS + m * P
            off = row0 - g * FD
            has_diag = 0 <= off <= L1_W - P
            tot[b] += cc[b] - (P if has_diag else 0)
            denom[b] += P * L1_W - (P if has_diag else 0)
    p_hat = tot / denom
    # p_hat should be increasing in b; enforce monotonicity for safety
    p_mono = np.maximum.accumulate(p_hat)
    if not (p_mono[0] <= p_off <= p_mono[-1]):
        return None  # bracket miss -> caller falls back to exact host path
    b_hi = int(np.searchsorted(p_mono, p_off, side="left"))
    if b_hi == 0:
        return float(DC2_GRID[0])
    b_lo = b_hi - 1
    p_lo, p_hi_v = p_mono[b_lo], p_mono[b_hi]
    frac = 0.0 if p_hi_v <= p_lo else (p_off - p_lo) / (p_hi_v - p_lo)
    return float(DC2_GRID[b_lo] + frac * (DC2_GRID[b_hi] - DC2_GRID[b_lo]))


def _host_fallback(data, rho_t, delta_t):
    """Pure-numpy reference path (only used if device assumptions break)."""
    data = np.asarray(data, np.float32)
    sq = np.sum(data * data, axis=1)
    d2 = sq[:, None] + sq[None, :] - 2.0 * (data @ data.T)
    dist = np.sqrt(np.maximum(d2, 0.0), dtype=np.float32)
    dc = np.percentile(dist, PCT)
    rho = np.exp(-((dist / dc) ** 2)).sum(axis=1).astype(np.float32)
    higher = rho[None, :] > rho[:, None]
    masked = np.where(higher, dist, np.inf)
    delta_m = masked.min(axis=1)
    nhd_m = masked.argmin(axis=1)
    has = higher.any(axis=1)
    delta = np.where(has, delta_m, dist.max(axis=1))
    nhd = np.where(has, nhd_m, np.arange(N))
    return _finish_labels(rho, delta, nhd, rho_t, delta_t)


def _finish_labels(rho, delta, nhd, rho_t, delta_t):
    is_center = (rho > rho_t) & (delta > delta_t)
    center_rank = np.cumsum(is_center.astype(np.int32)) - 1
    labels = np.where(is_center, center_rank, -1).astype(np.int32)
    order = np.argsort(-rho, kind="stable")
    for i in order:
        if labels[i] < 0:
            labels[i] = labels[nhd[i]]
    return labels


def kernel(data, rho_threshold, delta_threshold):
    data = np.ascontiguousarray(np.asarray(data, dtype=np.float32))
    assert data.shape == (N, D)
    rho_t = float(np.asarray(rho_threshold))
    delta_t = float(np.asarray(delta_threshold))

    lab = _kernel_cert(data, rho_t, delta_t)
    if lab is not None:
        return lab
    return _kernel_full(data, rho_t, delta_t)


def _kernel_cert(data, rho_t, delta_t):
    """All-centers fast path.

    Device computes, over every unordered point pair, either an exact
    (bf16-accurate) min of d2 or a sum-of-exp(-d2/2) close-pair detector.
    If no pair is closer than delta_threshold (plus error margin) and the
    rho lower bound 1 + (N-1)exp(-d2max/dc^2) clears rho_threshold, then
    every point satisfies rho > rho_t and delta > delta_t, making every
    point a cluster center: labels == arange exactly. Returns None when
    the certificate does not hold (caller falls back to the full path).
    """
    import ml_dtypes

    bf = ml_dtypes.bfloat16
    f8 = ml_dtypes.float8_e4m3fn
    sq = np.einsum("ij,ij->i", data, data, dtype=np.float32).astype(np.float32)
    sq64 = sq.astype(np.float64)

    # fp8 V rows [68, N]: x.T (64) + 3-way sq split + zero pad row
    s0 = sq.astype(f8)
    r1 = sq - s0.astype(np.float32)
    s1 = r1.astype(f8)
    s2 = (r1 - s1.astype(np.float32)).astype(f8)
    v8_rows = np.zeros((2 * KI8, N), f8)
    v8_rows[0:D] = data.astype(f8).T
    v8_rows[D] = s0
    v8_rows[D + 1] = s1
    v8_rows[D + 2] = s2
    pad_col = np.zeros((2 * KI8, 1), f8)
    pad_col[D:D + 3] = F8_PAD  # sentinel: pad d2 ~ 720, never min/detected

    sqh = sq.astype(bf)
    sql = (sq - sqh.astype(np.float32)).astype(bf)

    in_maps = []
    for c in range(NCORES):
        # shifted fp8 V: local col j = global col j + 128c, tail = sentinel
        sh = 128 * c
        uv8c = np.empty((2 * KI8, N + ROWS), f8)
        uv8c[:, 0:N - sh] = v8_rows[:, sh:]
        uv8c[:, N - sh:N] = pad_col
        uvdc = np.empty((CK, 2 * ROWS), bf)
        bias = np.empty((P, RB), np.float32)
        for m in range(RB):
            b = 8 * m + c
            rows = slice(b * P, (b + 1) * P)
            cols = slice(m * P, (m + 1) * P)
            u8blk = uv8c[:, N + m * P:N + (m + 1) * P]
            u8blk[0:D] = (-2.0 * data[rows]).astype(f8).T
            u8blk[D:D + 3] = 1.0
            u8blk[D + 3:] = 0.0
            uvdc[0:D, cols] = (-2.0 * data[rows]).astype(bf).T
            uvdc[D:CK, cols] = 1.0
            uvdc[0:D, ROWS + m * P:ROWS + (m + 1) * P] = data[rows].astype(bf).T
            uvdc[D, ROWS + m * P:ROWS + (m + 1) * P] = sqh[rows]
            uvdc[D + 1, ROWS + m * P:ROWS + (m + 1) * P] = sql[rows]
            bias[:, m] = -0.5 * sq[rows]
        in_maps.append(
            {
                "uv8": np.ascontiguousarray(
                    uv8c.reshape(2, KI8, N + ROWS).transpose(1, 0, 2)
                ),
                "uvd": np.ascontiguousarray(uvdc),
                "bias": bias,
            }
        )
    r = _run("cert", in_maps)

    plan = _plan_cert()
    m_glob = np.inf       # min over DVE-covered pairs of measured d2
    s_res_max = -np.inf   # max per-row fp8 exp-detector sum
    dg_min = np.inf       # min over intra-block off-diag pairs, measured d2
    dg_self_err = 0.0
    import ml_dtypes as _mld

    with np.errstate(divide="ignore"):
        for c in range(NCORES):
            out = r.results[c]["out"]  # [P, 548] f32
            S = out[:, 0:C_NACT].astype(np.float64)
            M = out[:, C_NACT:C_NACT + C_NDVE]
            G = np.ascontiguousarray(out[:, 36:548]).view(
                _mld.bfloat16
            ).astype(np.float64)  # [P, ROWS] exp vals
            for m in range(RB):
                rows = slice((8 * m + c) * P, (8 * m + c + 1) * P)
                scols = [t["col"] for t in plan
                         if t["m"] == m and t["eng"] == "A"]
                vcols = [t["col"] for t in plan
                         if t["m"] == m and t["eng"] == "V"]
                s_res_max = max(s_res_max, float(S[:, scols].sum(1).max()))
                if vcols:
                    mv = M[:, vcols].min(axis=1) + sq64[rows]
                    m_glob = min(m_glob, float(mv.min()))
                # diag sub-block: E = exp(-psum/2), d2 = -2 ln E + sq_i
                E = G[:, m * P:(m + 1) * P]
                d2m = -2.0 * np.log(np.maximum(E, 1e-300)) + sq64[rows][:, None]
                d2m[E <= 0.0] = np.inf  # exp underflow => pair is far
                self_d2 = np.diagonal(d2m)
                dg_self_err = max(dg_self_err, float(np.abs(self_d2).max()))
                np.fill_diagonal(d2m, np.inf)
                dg_min = min(dg_min, float(d2m.min()))

    if not np.isfinite(m_glob) or not np.isfinite(s_res_max):
        return None
    if s_res_max >= CERT_SMAX8 or dg_self_err >= CERT_EPSD:
        return None
    # DVE-covered pairs: true d2 > m_glob - eps8. Un-flagged ACT rows have
    # every term exp(-d2_meas/2) < SMAX8, so measured d2 > -2 ln SMAX8 = 7,
    # true d2 > CERT_DET8. Intra-block pairs: true d2 > dg_min - eps_diag.
    d2_lo = min(m_glob - CERT_EPS8, CERT_DET8, dg_min - CERT_EPSD)
    if d2_lo <= 0.0:
        return None
    # delta_i >= min_j dist > delta_t for every i
    if delta_t >= 0.0 and d2_lo <= delta_t * delta_t:
        return None
    # rho_i >= 1 + (N-1) exp(-d2max / dc^2), dc^2 >= d2_lo,
    # d2max <= (2 max|x|)^2 exactly on host
    d2max = float(4.0 * sq64.max())
    rho_lb = 1.0 + 0.9 * (N - 1) * float(np.exp(-d2max / d2_lo))
    if rho_t >= rho_lb:
        return None
    return np.arange(N, dtype=np.int32)


def _kernel_full(data, rho_t, delta_t):
    U, V, sq = _augmented(data)
    VT = V.T  # [K, N]

    # ---- L12: counts -> on-device dc^2 -> rho (single launch) ----------
    thr = np.broadcast_to(
        (SIG_ALPHA * DC2_GRID).astype(np.float32)[None, :], (P, NT)
    ).copy()
    tvec = DC2_GRID.astype(np.float32).reshape(1, NT)
    dvec = _cv_corrections(sq)
    uvc = np.ascontiguousarray(np.concatenate([U[0:ROWS].T, VT], axis=1)[:, 0:ROWS])
    in_maps = [
        {
            "uv": np.ascontiguousarray(
                np.concatenate([U[c * ROWS:(c + 1) * ROWS].T, VT], axis=1)
            ),
            "uvc": uvc,
            "thr": thr,
            "tvec": tvec,
            "dvec": dvec,
        }
        for c in range(NCORES)
    ]
    r12 = _run("l12", in_maps)

    # validate the on-device dc interpolation from the counts output
    q = r12.results[0]["counts"].astype(np.float64).sum(axis=0) - dvec[0].astype(
        np.float64
    )
    brackets = [
        b for b in range(NT - 1) if q[b] <= CSTAR < q[b + 1]
    ]
    if len(brackets) != 1 or not np.all(np.diff(q) > 0):
        return _host_fallback(data, rho_t, delta_t)

    rho = np.empty(N, np.float32)
    for c in range(NCORES):
        out = r12.results[c]["rho"]  # [P, RB]
        rho[c * ROWS:(c + 1) * ROWS] = out.T.reshape(-1)
    if not np.all(np.isfinite(rho)) or rho.min() < 0.5 or rho.max() > N + 1:
        return _host_fallback(data, rho_t, delta_t)

    # ---- host: sort by rho desc; prefix cutoffs ------------------------
    order = np.argsort(-rho, kind="stable")
    rho_sorted = rho[order]
    # c_i = #points with rho strictly greater (ties excluded)
    cuts = np.searchsorted(-rho_sorted, -rho_sorted, side="left").astype(np.int64)

    data_p = data[order]
    sq_p = sq[order]
    Up = U[order]
    Vp = V[order]
    rhs_p = np.ascontiguousarray(Vp.T)

    # round-robin block interleave: core c <- sorted blocks 8m + c
    NB = N // P  # 64 sorted row-blocks
    blk_rows = np.arange(N).reshape(NB, P)
    core_rows = [blk_rows[np.arange(RB) * NCORES + c].reshape(-1) for c in range(NCORES)]

    iota_in = np.broadcast_to(
        np.arange(WW, dtype=np.float32)[None, :], (P, WW)
    ).copy()
    in_maps = []
    for c in range(NCORES):
        rows = core_rows[c]
        cutrel = np.empty((P, RB), np.float32)
        for m in range(RB):
            base = (m // 2) * FD + WW * (m % 2)
            cutrel[:, m] = np.clip(cuts[rows[m * P:(m + 1) * P]] - base, 0, WW)
        in_maps.append(
            {
                "uv": np.ascontiguousarray(
                    np.concatenate([Up[rows].T, rhs_p], axis=1)
                ),
                "cut": cutrel,
                "iota": iota_in,
            }
        )
    r3 = _run("l3", in_maps)
    # dmin[i] holds per-source minima; dcol[k] = (col_base, col_len) of source k
    dmin = np.full((N, NCOL), np.inf, np.float32)
    for c in range(NCORES):
        out = r3.results[c]["dmin"]  # [P, RB*NCOL]
        rows = core_rows[c]
        for m in range(RB):
            gb = m // 2
            w_lo = WW * (m % 2)
            blk = rows[m * P:(m + 1) * P]
            for g in range(gb):
                dmin[blk, g] = out[:, m * NCOL + g]
            if w_lo > 0:
                dmin[blk, gb] = out[:, m * NCOL + gb]
            dmin[blk, G] = out[:, m * NCOL + G]

    # ---- host: delta, fallback rows, centers, nhd (lazy), labels -------
    delta2_sorted = dmin.min(axis=1)

    # rho-tie rows whose cutoff dips below their block's boundary group: the
    # device's full-group reduce included a few extra columns; fix exactly.
    win_base = ((np.arange(N) // P) // NCORES) * WW  # 1024*m per sorted row
    straddle_fix = {}
    for i in np.nonzero(cuts < win_base)[0]:
        cut = int(cuts[i])
        if cut == 0:
            delta2_sorted[i] = np.inf
            continue
        d2row = sq_p[i] + sq_p[:cut] - 2.0 * (data_p[:cut] @ data_p[i])
        j = int(np.argmin(d2row))
        delta2_sorted[i] = d2row[j]
        straddle_fix[i] = j

    empty = delta2_sorted >= EMPTY_SENTINEL  # no higher-density point
    delta_sorted = np.sqrt(np.maximum(delta2_sorted, 0.0), dtype=np.float32)
    for i in np.nonzero(empty)[0]:
        d2row = sq_p[i] + sq_p - 2.0 * (data_p @ data_p[i])
        delta_sorted[i] = np.sqrt(max(float(np.max(np.maximum(d2row, 0.0))), 0.0))

    delta = np.empty(N, np.float32)
    delta[order] = delta_sorted

    is_center = (rho > rho_t) & (delta > delta_t)
    center_rank = np.cumsum(is_center.astype(np.int32)) - 1
    labels = np.where(is_center, center_rank, -1).astype(np.int32)

    need_nhd = ~is_center[order]  # sorted positions whose label must propagate
    nhd = np.arange(N, dtype=np.int64)  # default: self (matches reference)
    for i in np.nonzero(need_nhd)[0]:
        if empty[i]:
            continue  # nhd stays self, as in reference
        if i in straddle_fix:
            nhd[order[i]] = order[straddle_fix[i]]
            continue
        k = int(np.argmin(dmin[i]))
        m = (i // P) // NCORES
        gb = m // 2
        w_lo = WW * (m % 2)
        if k == G:
            c0, clen = gb * FD + w_lo, WW
        elif k == gb:
            c0, clen = gb * FD, w_lo
        else:
            c0, clen = k * FD, FD
        end_local = int(np.clip(cuts[i] - c0, 0, clen))
        cols = slice(c0, c0 + end_local)
        d2part = sq_p[i] + sq_p[cols] - 2.0 * (data_p[cols] @ data_p[i])
        j_local = int(np.argmin(d2part))
        nhd[order[i]] = order[c0 + j_local]

    for i in order:
        if labels[i] < 0:
            labels[i] = labels[nhd[i]]
    return labels.astype(np.int32)

